# revision 2
# baseline (speedup 1.0000x reference)
"""BitLinear inference kernel for Trainium2: y = (x @ W_q^T) * s + bias.

Shapes: x [8192, 4096] f32, w_q [4096, 4096] ternary {-1,0,1}, s [1] f32,
bias [4096] f32 -> y [8192, 4096] f32.

Strategy:
- Data-parallel across 8 NeuronCores: each core computes 1024 of the 8192
  token rows against the full weight matrix (replicated W is only 8.4 MB
  as bf16, so replication is cheaper than replicating the 134 MB x).
- bf16 matmul with fp32 PSUM accumulation. The ternary weights are exact
  in bf16; only x rounds (2^-9 relative), giving ~1e-3 rel error overall.
  bf16 runs the PE at 4x the fp32 rate.
- Per-tensor scale s is folded into x on the host during the bf16 cast.
- On-core layout: output computed transposed (y^T tiles [n=128, m=512])
  so the per-output-channel bias lands on partitions; a single
  ScalarE/VectorE op per tile does PSUM->SBUF eviction + bias add.
- W tiles are the stationary PE operand (each LDWEIGHTS is reused by 2
  matmuls), x^T stays resident in SBUF, W streams through exactly once.
"""

import numpy as np
import ml_dtypes

M_TOTAL = 8192
D_IN = 4096
D_OUT = 4096
N_CORES = 8
P = 128
M = M_TOTAL // N_CORES  # 1024 tokens per core
KO = D_IN // P          # 32 contraction tiles
NT = D_OUT // P         # 32 output-channel tiles
MF = 512                # matmul moving free dim (= one fp32 PSUM bank)
MC = M // MF            # 2 m-chunks per core

_CACHE = {}


def build_nc(repeats=1):
    """Build + compile the per-core Bass module.

    repeats > 1 wraps the compute in a hardware loop that recomputes the
    (identical) output that many times -- used only for timing runs.
    """
    import concourse.mybir as mybir
    import concourse.tile as tile
    from concourse import bacc

    nc = bacc.Bacc(
        "TRN2",
        target_bir_lowering=False,
        debug=False,
        num_devices=N_CORES,
    )
    bf16 = mybir.dt.bfloat16
    f32 = mybir.dt.float32

    xt = nc.dram_tensor("xt", [D_IN, M], bf16, kind="ExternalInput")
    wt = nc.dram_tensor("wt", [NT, P, KO, P], bf16, kind="ExternalInput")
    bt = nc.dram_tensor("bt", [P, NT], f32, kind="ExternalInput")
    yt = nc.dram_tensor("yt", [D_OUT, M], f32, kind="ExternalOutput")

    xt_r = xt.ap().rearrange("(ko p) m -> p ko m", p=P)
    yt_r = yt.ap().rearrange("(nt p) m -> p nt m", p=P)
    wt_a = wt.ap()
    ident = mybir.ActivationFunctionType.Identity

    with tile.TileContext(nc) as tc:
        with (
            tc.tile_pool(name="xpool", bufs=1) as xpool,
            tc.tile_pool(name="cpool", bufs=1) as cpool,
            tc.tile_pool(name="wpool", bufs=3) as wpool,
            tc.tile_pool(name="opool", bufs=4) as opool,
            tc.tile_pool(name="pspool", bufs=2, space="PSUM") as pspool,
        ):
            bias_sb = cpool.tile([P, NT], f32, tag="bias")
            nc.sync.dma_start(bias_sb[:], bt.ap())

            xs = []
            for ko in range(KO):
                xk = xpool.tile([P, M], bf16, tag=f"x{ko}")
                nc.sync.dma_start(xk[:], xt_r[:, ko, :])
                xs.append(xk)

            def body(_iv=None):
                for nt in range(NT):
                    w_sb = wpool.tile([P, KO, P], bf16, tag="w")
                    nc.sync.dma_start(w_sb[:], wt_a[nt])
                    pss = [
                        pspool.tile([P, MF], f32, tag=f"ps{mc}", name=f"ps{mc}")
                        for mc in range(MC)
                    ]
                    for ko in range(KO):
                        lhsT = w_sb[:, ko, :]
                        for mc in range(MC):
                            nc.tensor.matmul(
                                pss[mc][:],
                                lhsT,
                                xs[ko][:, mc * MF:(mc + 1) * MF],
                                start=(ko == 0),
                                stop=(ko == KO - 1),
                            )
                    for mc in range(MC):
                        o_sb = opool.tile([P, MF], f32, tag=f"o{mc}")
                        bias_col = bias_sb[:, nt:nt + 1]
                        if mc == 0:
                            nc.scalar.activation(
                                o_sb[:], pss[mc][:], ident,
                                bias=bias_col, scale=1.0,
                            )
                        else:
                            nc.vector.tensor_scalar_add(
                                o_sb[:], pss[mc][:], bias_col,
                            )
                        nc.sync.dma_start(
                            yt_r[:, nt, mc * MF:(mc + 1) * MF], o_sb[:],
                        )

            if repeats == 1:
                body()
            else:
                with tc.For_i(0, repeats, 1) as iv:
                    body(iv)

    nc.compile()
    return nc


def prep_inputs(x, w_q, s, bias):
    bf16 = ml_dtypes.bfloat16
    s_val = np.float32(np.asarray(s).reshape(-1)[0])
    w = np.asarray(w_q)
    # wt[nt, kp, ko, nn] = w[nt*128 + nn, ko*128 + kp]
    wt = np.ascontiguousarray(
        w.reshape(NT, P, KO, P).transpose(0, 3, 2, 1).astype(bf16)
    )
    bt = np.ascontiguousarray(
        np.asarray(bias, dtype=np.float32).reshape(NT, P).T
    )
    in_maps = []
    for c in range(N_CORES):
        xc = np.asarray(x[c * M:(c + 1) * M], dtype=np.float32) * s_val
        xt = np.ascontiguousarray(xc.T.astype(bf16))  # [D_IN, M]
        in_maps.append({"xt": xt, "wt": wt, "bt": bt})
    return in_maps


def run(nc, in_maps, **kwargs):
    from concourse import bass_utils

    return bass_utils.run_bass_kernel_spmd(
        nc, in_maps, core_ids=list(range(N_CORES)), **kwargs
    )


def kernel(x, w_q, s, bias):
    nc = _CACHE.get("nc")
    if nc is None:
        nc = _CACHE["nc"] = build_nc()
    in_maps = prep_inputs(x, w_q, s, bias)
    res = run(nc, in_maps)
    y = np.empty((M_TOTAL, D_OUT), dtype=np.float32)
    for c in range(N_CORES):
        y[c * M:(c + 1) * M] = res.results[c]["yt"].T
    return y
